# revision 20
# baseline (speedup 1.0000x reference)
"""MetaDGCRU Trainium2 kernel.

Problem (hardcoded shapes): B=8, N=400, INPUT_DIM=2, HIDDEN=64,
GRAPH_NUM=2, HOP_K=2, NODE_EMB_DIM=16, IN_FEAT=66, I_DIM=330.

Sharding: data-parallel over batch B across the 8 NeuronCores (one batch
element per core); weight pools replicated, per-graph adjacencies sharded
with their batch.

Per-core computation (feature-on-partition / "transposed" layouts):
  xsT = [x;state].T                                    [66, 400]
  hops transposed-out:  YT = lhsT(X_nat).T @ AT        (PE, 4 m-chunks)
  hT = concat pieces -> 3 tiles of [128, 400] (i padded 330->384)
  gT[(d,i), n] = embT[d,n] * hT[i,n]                   (DVE+GPS, 48 chunks)
  zrT = bias(start=True, K=16) + sum_c Wg[c].T @ gT[c] (PE, 48 + 1 MMs)
  z,r = sigmoid(zrT);  xrsT = [xT; rT*stateT];  repeat -> hcT = tanh(...)
  out hT = hcT + zT*(stateT - hcT)                     [64, 400] f32

DMA strategy: HWDGE rings are FIFO per engine, so ordering is by emission:
the SP ring streams adjacency first, then embrep/Wg quarters interleaved in
the order compute consumes them; the ACT ring carries the small constants
and the mid-kernel piece/shift DMAs so they never queue behind bulk weights.
"""

import os

os.environ.setdefault("MYCRO_LOCAL_CACHE", "1")

import numpy as np
import ml_dtypes

B, N = 8, 400
INPUT_DIM, HIDDEN = 2, 64
GRAPH_NUM, HOP_K = 2, 2
D_EMB = 16
IN_FEAT = INPUT_DIM + HIDDEN               # 66
I_DIM = (GRAPH_NUM * HOP_K + 1) * IN_FEAT  # 330
KCH = 3                                    # i-chunks per d (128 each)
I_PAD = KCH * 128                          # 384
NCH = D_EMB * KCH                          # 48 total K chunks
O_G = 2 * HIDDEN                           # 128 gate out (z|r)
O_C = HIDDEN                               # 64 candidate out
NPAD = 512                                 # node dim padded for clean DMA packing

BF16 = ml_dtypes.bfloat16
MCHUNKS = [(0, 128), (128, 128), (256, 128), (384, 16)]  # node-dim chunking
QD = 4                                     # d's per streaming quarter

GPS_EVERY = 3  # every 3rd gT-build op runs on GpSimd instead of DVE

_CACHE = {}


def _emit(nc, tc, tile, mybir, ctx):
    """Emit the per-core kernel into TileContext tc."""
    dt = mybir.dt
    Sig = mybir.ActivationFunctionType.Sigmoid
    Tanh = mybir.ActivationFunctionType.Tanh
    Copy = mybir.ActivationFunctionType.Copy

    d_at = nc.dram_tensor("at", [GRAPH_NUM, 128, 4 * N], dt.bfloat16, kind="ExternalInput")
    d_xsT = nc.dram_tensor("xsT", [IN_FEAT, N], dt.bfloat16, kind="ExternalInput")
    d_xsnat = nc.dram_tensor("xsnat", [128, 4 * IN_FEAT], dt.bfloat16, kind="ExternalInput")
    d_state2 = nc.dram_tensor("state2", [2 * HIDDEN, N], dt.float32, kind="ExternalInput")
    d_embT = nc.dram_tensor("embT", [D_EMB, N], dt.bfloat16, kind="ExternalInput")
    d_embrep = nc.dram_tensor("embrep", [128, D_EMB * N], dt.bfloat16, kind="ExternalInput")
    d_wg = nc.dram_tensor("wg", [128, NCH * O_G], dt.bfloat16, kind="ExternalInput")
    d_wc = nc.dram_tensor("wc", [128, NCH * O_C], dt.bfloat16, kind="ExternalInput")
    d_bg = nc.dram_tensor("bg", [D_EMB, O_G], dt.bfloat16, kind="ExternalInput")
    d_bc = nc.dram_tensor("bc", [D_EMB, O_C], dt.bfloat16, kind="ExternalInput")
    d_ident = nc.dram_tensor("ident", [128, 128], dt.bfloat16, kind="ExternalInput")
    d_out = nc.dram_tensor("out", [HIDDEN, N], dt.float32, kind="ExternalOutput")

    cpool = ctx.enter_context(tc.tile_pool(name="const", bufs=1))
    hpool = ctx.enter_context(tc.tile_pool(name="hbuf", bufs=1))
    gpool = ctx.enter_context(tc.tile_pool(name="gbuf", bufs=1))
    spool = ctx.enter_context(tc.tile_pool(name="small", bufs=2))
    ppool = ctx.enter_context(tc.tile_pool(name="psum", bufs=2, space="PSUM"))
    ptp = ctx.enter_context(tc.tile_pool(name="psumT", bufs=2, space="PSUM"))
    pzr = ctx.enter_context(tc.tile_pool(name="psumZR", bufs=1, space="PSUM"))

    # ---- SP-ring priority inputs (FIFO: first emitted = first transferred) ----
    at_sb = []
    for g in range(GRAPH_NUM):
        t = cpool.tile([128, 4 * N], dt.bfloat16, name=f"at{g}")
        nc.sync.dma_start(t[:], d_at[g, :, :])
        at_sb.append(t)
    xsnat_sb = cpool.tile([128, 4 * IN_FEAT], dt.bfloat16, name="xsnat")
    nc.sync.dma_start(xsnat_sb[:], d_xsnat[:, :])

    # hT tiles + first pieces
    hT_g = [hpool.tile([128, N], dt.bfloat16, name=f"hTg{t}") for t in range(KCH)]
    hT_c = [hpool.tile([128, N], dt.bfloat16, name=f"hTc{t}") for t in range(KCH)]
    nc.vector.memset(hT_g[2][:, :], 0.0)
    nc.vector.memset(hT_c[2][:, :], 0.0)
    nc.sync.dma_start(hT_g[0][0:IN_FEAT, :], d_xsT[:, :])
    nc.sync.dma_start(hT_c[0][0:INPUT_DIM, :], d_xsT[0:INPUT_DIM, :])

    # ---- SP-ring bulk stream, interleaved in consumption order ----
    wg_sb = cpool.tile([128, NCH * O_G], dt.bfloat16, name="wg")
    wc_sb = cpool.tile([128, NCH * O_C], dt.bfloat16, name="wc")
    for q in range(D_EMB // QD):
        w0 = q * QD * KCH * O_G
        nc.sync.dma_start(wg_sb[:, w0:w0 + QD * KCH * O_G],
                          d_wg[:, w0:w0 + QD * KCH * O_G])

    # ---- ACT-ring small constants (separate HW queue from the bulk stream) ----
    embT_sb = cpool.tile([D_EMB, N], dt.bfloat16, name="embT")
    nc.scalar.dma_start(embT_sb[:], d_embT[:, :])
    ident_sb = cpool.tile([128, 128], dt.bfloat16, name="ident")
    nc.scalar.dma_start(ident_sb[:], d_ident[:, :])
    bg_sb = cpool.tile([D_EMB, O_G], dt.bfloat16, name="bg")
    nc.scalar.dma_start(bg_sb[:], d_bg[:, :])
    bc_sb = cpool.tile([D_EMB, O_C], dt.bfloat16, name="bc")
    nc.scalar.dma_start(bc_sb[:], d_bc[:, :])

    # embrep + wc + state2 streamed on the ACT HWDGE queue
    embrep_sb = cpool.tile([128, D_EMB * N], dt.bfloat16, name="embrep")
    for q in range(D_EMB // QD):
        e0 = q * QD * N
        nc.scalar.dma_start(embrep_sb[:, e0:e0 + QD * N], d_embrep[:, e0:e0 + QD * N])
    for h in range(2):
        w0 = h * (NCH // 2) * O_C
        nc.scalar.dma_start(wc_sb[:, w0:w0 + (NCH // 2) * O_C],
                            d_wc[:, w0:w0 + (NCH // 2) * O_C])
    state2_sb = cpool.tile([2 * HIDDEN, N], dt.float32, name="state2")
    nc.scalar.dma_start(state2_sb[:], d_state2[:, :])

    # dummy matmuls warm the PE (HAM) during the adjacency DMA wait
    ones_sb = cpool.tile([128, 512], dt.bfloat16, name="ones_sb")
    nc.vector.memset(ones_sb[:, :], 1.0)
    pbc = ctx.enter_context(tc.tile_pool(name="psumBC", bufs=2, space="PSUM"))
    for w in range(14):
        warm_ps = pbc.tile([128, 512], dt.float32, name=f"warm_ps{w}", tag="warmps")
        nc.tensor.matmul(warm_ps[:], ones_sb[:, 0:128], ones_sb[:, :],
                         start=True, stop=True)

    # warm the ACT table sets early so the loads overlap the input DMAs
    warm = hpool.tile([1, 8], dt.float32, name="warm")
    nc.vector.memset(warm[:, :], 0.0)
    nc.scalar.activation(warm[:, 0:4], warm[:, 4:8], Copy)
    nc.scalar.activation(warm[:, 0:4], warm[:, 4:8], Sig)
    nc.scalar.activation(warm[:, 0:4], warm[:, 4:8], Tanh)

    # gT buffer: 48 chunks of [128, N] side by side (shared gate/cand)
    gT = gpool.tile([128, NCH * N], dt.bfloat16, name="gT")

    def piece_to_hT(hT, piece, p_idx):
        """DMA piece [IN_FEAT, N] into hT tiles at global row 66*p_idx
        (ACT ring: emitted right after the producing ACT copy)."""
        r0 = IN_FEAT * p_idx
        t0, o0 = divmod(r0, 128)
        t1 = (r0 + IN_FEAT - 1) // 128
        if t0 == t1:
            nc.sync.dma_start(hT[t0][o0:o0 + IN_FEAT, :], piece[:, :])
        else:
            n0 = 128 - o0
            nc.sync.dma_start(hT[t0][o0:128, :], piece[0:n0, :])
            nc.sync.dma_start(hT[t1][0:IN_FEAT - n0, :], piece[n0:IN_FEAT, :])

    def hop(lhsT_of, g, name):
        """One propagation Y = A_g @ X, transposed out. lhsT_of(k)->AP [mlen,66]."""
        yt_ps = ppool.tile([IN_FEAT, N], dt.float32, name=f"ps_{name}", tag="hopps")
        for k, (moff, mlen) in enumerate(MCHUNKS):
            nc.tensor.matmul(
                yt_ps[:], lhsT_of(k), at_sb[g][0:mlen, k * N:(k + 1) * N],
                start=(k == 0), stop=(k == len(MCHUNKS) - 1),
            )
        yt = spool.tile([IN_FEAT, N], dt.bfloat16, name=f"yt_{name}", tag="hopsb")
        nc.scalar.activation(yt[:], yt_ps[:], Copy)
        return yt

    def nat_slicer(tl):
        return lambda k: tl[0:MCHUNKS[k][1], k * IN_FEAT:(k + 1) * IN_FEAT]

    def naturalize(yt, name):
        """PE-transpose YT [66, N] -> natural tile [128, 4*66]."""
        natt = spool.tile([128, 4 * IN_FEAT], dt.bfloat16, name=f"nat_{name}", tag="natsb")
        for k, (moff, mlen) in enumerate(MCHUNKS):
            tp = ptp.tile([mlen, IN_FEAT], dt.bfloat16, name=f"tp_{name}{k}", tag="trps")
            nc.tensor.transpose(tp[:], yt[:, moff:moff + mlen], ident_sb[0:IN_FEAT, 0:IN_FEAT])
            nc.scalar.activation(natt[0:mlen, k * IN_FEAT:(k + 1) * IN_FEAT], tp[:], Copy)
        return natt

    def meta_phase(hT, lhsT_of, w_sb, b_sb, o_dim, psum_out, phase):
        """Hops + gT build + meta matmul, accumulating into psum_out [o_dim, N]."""
        for g in range(GRAPH_NUM):
            y1t = hop(lhsT_of, g, f"{phase}y1g{g}")
            piece_to_hT(hT, y1t, 1 + 2 * g)
            y1nat = naturalize(y1t, f"{phase}g{g}")
            y2t = hop(nat_slicer(y1nat), g, f"{phase}y2g{g}")
            piece_to_hT(hT, y2t, 2 + 2 * g)

        # bias matmul resets PSUM
        nc.tensor.matmul(psum_out[:], b_sb[:], embT_sb[:], start=True, stop=False)

        # gT build (fused 4-d DVE ops) + accumulate matmuls; quarter-major
        # (DMA stream order), k inner so early hT tiles are consumed first
        for q in range(D_EMB // QD):
            for k in range(KCH):
                d0 = q * QD
                c0 = d0 * KCH + k
                # out chunks c = (d0+j)*KCH + k, j=0..3 -> stride KCH*N
                out_ap = (gT[:, c0 * N:(c0 + KCH * (QD - 1) + 1) * N]
                          .rearrange("p (c n) -> p c n", n=N)[:, ::KCH, :])
                in0 = (hT[k][:, :].rearrange("p (u n) -> p u n", u=1)
                       .broadcast_to([128, QD, N]))
                in1 = (embrep_sb[:, d0 * N:(d0 + QD) * N]
                       .rearrange("p (c n) -> p c n", n=N))
                nc.vector.tensor_tensor(out_ap, in0, in1, mybir.AluOpType.mult)
                for j in range(QD):
                    c = (d0 + j) * KCH + k
                    nc.tensor.matmul(
                        psum_out[:],
                        w_sb[:, c * o_dim:(c + 1) * o_dim],
                        gT[:, c * N:(c + 1) * N],
                        start=False,
                        stop=(q == D_EMB // QD - 1 and k == KCH - 1 and j == QD - 1),
                    )

    # ================= gate phase =================
    zr_ps = pzr.tile([O_G, N], dt.float32, name="zr_ps")
    meta_phase(hT_g, nat_slicer(xsnat_sb), wg_sb, bg_sb, O_G, zr_ps, "g")
    zr_sig = hpool.tile([O_G, N], dt.float32, name="zr_sig")
    nc.scalar.activation(zr_sig[:], zr_ps[:], Sig)

    # xrsT state part: rows 64:128 of zr_sig are rT; state2 rows 64:128 = stateT
    rs_scratch = hpool.tile([O_G, N], dt.bfloat16, name="rs_scratch")
    nc.vector.tensor_mul(rs_scratch[HIDDEN:O_G, :], zr_sig[HIDDEN:O_G, :],
                         state2_sb[HIDDEN:O_G, :])
    nc.sync.dma_start(hT_c[0][INPUT_DIM:IN_FEAT, :], rs_scratch[HIDDEN:O_G, :])

    # xrs natural (lhsT for candidate first hops)
    xrsnat = naturalize(hT_c[0][0:IN_FEAT, :], "xrs")

    # ================= candidate phase =================
    hc_ps = pzr.tile([O_C, N], dt.float32, name="hc_ps")
    meta_phase(hT_c, nat_slicer(xrsnat), wc_sb, bc_sb, O_C, hc_ps, "c")
    hc_t = hpool.tile([O_C, N], dt.float32, name="hc_t")
    nc.scalar.activation(hc_t[:], hc_ps[:], Tanh)

    # ================= output blend =================
    # h = hc + z*(state - hc)
    d1 = hpool.tile([O_C, N], dt.float32, name="d1")
    nc.vector.tensor_sub(d1[:], state2_sb[0:HIDDEN, :], hc_t[:])
    d2 = hpool.tile([O_C, N], dt.float32, name="d2")
    nc.vector.tensor_mul(d2[:], zr_sig[0:HIDDEN, :], d1[:])
    hout = hpool.tile([O_C, N], dt.float32, name="hout")
    nc.vector.tensor_add(hout[:], hc_t[:], d2[:])
    nc.sync.dma_start(d_out[:, :], hout[:])


def _build_nc():
    import concourse.tile as tile
    import concourse.mybir as mybir
    from contextlib import ExitStack
    from concourse import bacc

    nc = bacc.Bacc(trn_type="TRN2")
    with tile.TileContext(nc) as tc:
        with ExitStack() as ctx:
            _emit(nc, tc, tile, mybir, ctx)
    nc.finalize()
    return nc


def _prep_core_inputs(b, x, state, graphs, node_emb, Wg, bg, Wc, bc):
    """Host-side shard + layout prep for core b. Layouts match SBUF tiles."""
    f32 = np.float32
    at = graphs[:, b].transpose(0, 2, 1)                         # [G, N, N] = A.T
    at_pad = np.zeros((GRAPH_NUM, NPAD, N), f32)
    at_pad[:, :N, :] = at
    at_pk = (at_pad.reshape(GRAPH_NUM, 4, 128, N)
             .transpose(0, 2, 1, 3)
             .reshape(GRAPH_NUM, 128, 4 * N))                    # [G,128,(k n)]
    xs = np.concatenate([x[b], state[b]], axis=-1)               # [N, 66] f32
    xsT = np.ascontiguousarray(xs.T).astype(BF16)                # [66, N]
    xs_pad = np.zeros((NPAD, IN_FEAT), f32)
    xs_pad[:N] = xs
    xsnat = (xs_pad.reshape(4, 128, IN_FEAT)
             .transpose(1, 0, 2)
             .reshape(128, 4 * IN_FEAT))                         # [128,(k f)]
    stT = np.ascontiguousarray(state[b].T.astype(f32))           # [64, N]
    state2 = np.concatenate([stT, stT], axis=0)                  # [128, N] f32
    embT = np.ascontiguousarray(node_emb[b].T).astype(BF16)      # [16, N]
    embrep = np.ascontiguousarray(np.broadcast_to(
        embT.reshape(1, D_EMB * N), (128, D_EMB * N)))           # [128, 16N]

    def pack_w(W, o_dim):
        # W [16, 330, o] -> [128, 48*o]; chunk c=(d,k): rows i=128k+p
        Wp = np.zeros((D_EMB, I_PAD, o_dim), np.float32)
        Wp[:, :I_DIM, :] = W
        Wp = Wp.reshape(D_EMB, KCH, 128, o_dim)                  # [d,k,p,o]
        Wp = Wp.transpose(2, 0, 1, 3).reshape(128, NCH * o_dim)  # [p,(d,k,o)]
        return np.ascontiguousarray(Wp).astype(BF16)

    ident = np.eye(128, dtype=np.float32).astype(BF16)
    return {
        "at": np.ascontiguousarray(at_pk).astype(BF16),
        "xsT": xsT,
        "xsnat": np.ascontiguousarray(xsnat).astype(BF16),
        "state2": state2,
        "embT": embT,
        "embrep": embrep,
        "wg": pack_w(Wg, O_G),
        "wc": pack_w(Wc, O_C),
        "bg": bg.astype(BF16),
        "bc": bc.astype(BF16),
        "ident": ident,
    }


def kernel_with_results(x, state, graphs, node_emb, Wg, bg, Wc, bc, trace=False):
    from concourse.bass_utils import run_bass_kernel_spmd

    x = np.asarray(x, np.float32)
    state = np.asarray(state, np.float32)
    graphs = np.asarray(graphs, np.float32)
    node_emb = np.asarray(node_emb, np.float32)
    Wg = np.asarray(Wg, np.float32)
    bg = np.asarray(bg, np.float32)
    Wc = np.asarray(Wc, np.float32)
    bc = np.asarray(bc, np.float32)

    if "nc" not in _CACHE:
        _CACHE["nc"] = _build_nc()
    nc = _CACHE["nc"]

    in_maps = [
        _prep_core_inputs(b, x, state, graphs, node_emb, Wg, bg, Wc, bc)
        for b in range(B)
    ]
    res = run_bass_kernel_spmd(nc, in_maps, core_ids=list(range(B)), trace=trace)
    out = np.stack(
        [np.ascontiguousarray(res.results[b]["out"].T) for b in range(B)], axis=0
    )  # [B, N, HIDDEN] f32
    return out, res


def kernel(**inputs):
    out, _ = kernel_with_results(**inputs)
    return out


# revision 21
# speedup vs baseline: 1.0127x; 1.0127x over previous
"""MetaDGCRU Trainium2 kernel.

Problem (hardcoded shapes): B=8, N=400, INPUT_DIM=2, HIDDEN=64,
GRAPH_NUM=2, HOP_K=2, NODE_EMB_DIM=16, IN_FEAT=66, I_DIM=330.

Sharding: data-parallel over batch B across the 8 NeuronCores (one batch
element per core); weight pools replicated, per-graph adjacencies sharded
with their batch.

Per-core computation (feature-on-partition / "transposed" layouts):
  xsT = [x;state].T                                    [66, 400]
  hops transposed-out:  YT = lhsT(X_nat).T @ AT        (PE, 4 m-chunks)
  hT = concat pieces -> 3 tiles of [128, 400] (i padded 330->384)
  gT[(d,i), n] = embT[d,n] * hT[i,n]                   (DVE+GPS, 48 chunks)
  zrT = bias(start=True, K=16) + sum_c Wg[c].T @ gT[c] (PE, 48 + 1 MMs)
  z,r = sigmoid(zrT);  xrsT = [xT; rT*stateT];  repeat -> hcT = tanh(...)
  out hT = hcT + zT*(stateT - hcT)                     [64, 400] f32

DMA strategy: HWDGE rings are FIFO per engine, so ordering is by emission:
the SP ring streams adjacency first, then embrep/Wg quarters interleaved in
the order compute consumes them; the ACT ring carries the small constants
and the mid-kernel piece/shift DMAs so they never queue behind bulk weights.
"""

import os

os.environ.setdefault("MYCRO_LOCAL_CACHE", "1")

import numpy as np
import ml_dtypes

B, N = 8, 400
INPUT_DIM, HIDDEN = 2, 64
GRAPH_NUM, HOP_K = 2, 2
D_EMB = 16
IN_FEAT = INPUT_DIM + HIDDEN               # 66
I_DIM = (GRAPH_NUM * HOP_K + 1) * IN_FEAT  # 330
KCH = 3                                    # i-chunks per d (128 each)
I_PAD = KCH * 128                          # 384
NCH = D_EMB * KCH                          # 48 total K chunks
O_G = 2 * HIDDEN                           # 128 gate out (z|r)
O_C = HIDDEN                               # 64 candidate out
NPAD = 512                                 # node dim padded for clean DMA packing

BF16 = ml_dtypes.bfloat16
MCHUNKS = [(0, 128), (128, 128), (256, 128), (384, 16)]  # node-dim chunking
QD = 4                                     # d's per streaming quarter

GPS_EVERY = 3  # every 3rd gT-build op runs on GpSimd instead of DVE

_CACHE = {}


def _emit(nc, tc, tile, mybir, ctx):
    """Emit the per-core kernel into TileContext tc."""
    dt = mybir.dt
    Sig = mybir.ActivationFunctionType.Sigmoid
    Tanh = mybir.ActivationFunctionType.Tanh
    Copy = mybir.ActivationFunctionType.Copy

    d_at = nc.dram_tensor("at", [GRAPH_NUM, 128, 4 * N], dt.bfloat16, kind="ExternalInput")
    d_xsT = nc.dram_tensor("xsT", [IN_FEAT, N], dt.bfloat16, kind="ExternalInput")
    d_xsnat = nc.dram_tensor("xsnat", [128, 4 * IN_FEAT], dt.bfloat16, kind="ExternalInput")
    d_state2 = nc.dram_tensor("state2", [2 * HIDDEN, N], dt.float32, kind="ExternalInput")
    d_embT = nc.dram_tensor("embT", [D_EMB, N], dt.bfloat16, kind="ExternalInput")
    d_embrep = nc.dram_tensor("embrep", [128, D_EMB * N], dt.bfloat16, kind="ExternalInput")
    d_wg = nc.dram_tensor("wg", [128, NCH * O_G], dt.bfloat16, kind="ExternalInput")
    d_wc = nc.dram_tensor("wc", [128, NCH * O_C], dt.bfloat16, kind="ExternalInput")
    d_bg = nc.dram_tensor("bg", [D_EMB, O_G], dt.bfloat16, kind="ExternalInput")
    d_bc = nc.dram_tensor("bc", [D_EMB, O_C], dt.bfloat16, kind="ExternalInput")
    d_ident = nc.dram_tensor("ident", [128, 128], dt.bfloat16, kind="ExternalInput")
    d_out = nc.dram_tensor("out", [HIDDEN, N], dt.float32, kind="ExternalOutput")

    cpool = ctx.enter_context(tc.tile_pool(name="const", bufs=1))
    hpool = ctx.enter_context(tc.tile_pool(name="hbuf", bufs=1))
    gpool = ctx.enter_context(tc.tile_pool(name="gbuf", bufs=1))
    spool = ctx.enter_context(tc.tile_pool(name="small", bufs=2))
    ppool = ctx.enter_context(tc.tile_pool(name="psum", bufs=2, space="PSUM"))
    ptp = ctx.enter_context(tc.tile_pool(name="psumT", bufs=2, space="PSUM"))
    pzr = ctx.enter_context(tc.tile_pool(name="psumZR", bufs=1, space="PSUM"))

    # ---- SP-ring priority inputs (FIFO: first emitted = first transferred) ----
    at_sb = []
    for g in range(GRAPH_NUM):
        t = cpool.tile([128, 4 * N], dt.bfloat16, name=f"at{g}")
        nc.sync.dma_start(t[:], d_at[g, :, :])
        at_sb.append(t)
    xsnat_sb = cpool.tile([128, 4 * IN_FEAT], dt.bfloat16, name="xsnat")
    nc.sync.dma_start(xsnat_sb[:], d_xsnat[:, :])

    # hT tiles + first pieces
    hT_g = [hpool.tile([128, N], dt.bfloat16, name=f"hTg{t}") for t in range(KCH)]
    hT_c = [hpool.tile([128, N], dt.bfloat16, name=f"hTc{t}") for t in range(KCH)]
    nc.vector.memset(hT_g[2][:, :], 0.0)
    nc.vector.memset(hT_c[2][:, :], 0.0)
    nc.sync.dma_start(hT_g[0][0:IN_FEAT, :], d_xsT[:, :])
    nc.sync.dma_start(hT_c[0][0:INPUT_DIM, :], d_xsT[0:INPUT_DIM, :])

    # ---- SP-ring bulk stream, in consumption order ----
    embrep_sb = cpool.tile([128, D_EMB * N], dt.bfloat16, name="embrep")
    wg_sb = cpool.tile([128, NCH * O_G], dt.bfloat16, name="wg")
    wc_sb = cpool.tile([128, NCH * O_C], dt.bfloat16, name="wc")
    for q in range(D_EMB // QD):
        e0 = q * QD * N
        nc.sync.dma_start(embrep_sb[:, e0:e0 + QD * N], d_embrep[:, e0:e0 + QD * N])
    for q in range(D_EMB // QD):
        w0 = q * QD * KCH * O_G
        nc.sync.dma_start(wg_sb[:, w0:w0 + QD * KCH * O_G],
                          d_wg[:, w0:w0 + QD * KCH * O_G])
    for h in range(2):
        w0 = h * (NCH // 2) * O_C
        nc.sync.dma_start(wc_sb[:, w0:w0 + (NCH // 2) * O_C],
                          d_wc[:, w0:w0 + (NCH // 2) * O_C])
    state2_sb = cpool.tile([2 * HIDDEN, N], dt.float32, name="state2")
    nc.sync.dma_start(state2_sb[:], d_state2[:, :])

    # ---- ACT-ring small constants (separate HW queue from the bulk stream) ----
    embT_sb = cpool.tile([D_EMB, N], dt.bfloat16, name="embT")
    nc.scalar.dma_start(embT_sb[:], d_embT[:, :])
    ident_sb = cpool.tile([128, 128], dt.bfloat16, name="ident")
    nc.scalar.dma_start(ident_sb[:], d_ident[:, :])
    bg_sb = cpool.tile([D_EMB, O_G], dt.bfloat16, name="bg")
    nc.scalar.dma_start(bg_sb[:], d_bg[:, :])
    bc_sb = cpool.tile([D_EMB, O_C], dt.bfloat16, name="bc")
    nc.scalar.dma_start(bc_sb[:], d_bc[:, :])


    # dummy matmuls warm the PE (HAM) during the adjacency DMA wait
    ones_sb = cpool.tile([128, 512], dt.bfloat16, name="ones_sb")
    nc.vector.memset(ones_sb[:, :], 1.0)
    pbc = ctx.enter_context(tc.tile_pool(name="psumBC", bufs=2, space="PSUM"))
    for w in range(10):
        warm_ps = pbc.tile([128, 512], dt.float32, name=f"warm_ps{w}", tag="warmps")
        nc.tensor.matmul(warm_ps[:], ones_sb[:, 0:128], ones_sb[:, :],
                         start=True, stop=True)

    # warm the ACT table sets early so the loads overlap the input DMAs
    warm = hpool.tile([1, 8], dt.float32, name="warm")
    nc.vector.memset(warm[:, :], 0.0)
    nc.scalar.activation(warm[:, 0:4], warm[:, 4:8], Copy)
    nc.scalar.activation(warm[:, 0:4], warm[:, 4:8], Sig)
    nc.scalar.activation(warm[:, 0:4], warm[:, 4:8], Tanh)

    # gT buffer: 48 chunks of [128, N] side by side (shared gate/cand)
    gT = gpool.tile([128, NCH * N], dt.bfloat16, name="gT")

    def piece_to_hT(hT, piece, p_idx):
        """DMA piece [IN_FEAT, N] into hT tiles at global row 66*p_idx
        (ACT ring: emitted right after the producing ACT copy)."""
        r0 = IN_FEAT * p_idx
        t0, o0 = divmod(r0, 128)
        t1 = (r0 + IN_FEAT - 1) // 128
        if t0 == t1:
            nc.scalar.dma_start(hT[t0][o0:o0 + IN_FEAT, :], piece[:, :])
        else:
            n0 = 128 - o0
            nc.scalar.dma_start(hT[t0][o0:128, :], piece[0:n0, :])
            nc.scalar.dma_start(hT[t1][0:IN_FEAT - n0, :], piece[n0:IN_FEAT, :])

    def hop(lhsT_of, g, name):
        """One propagation Y = A_g @ X, transposed out. lhsT_of(k)->AP [mlen,66]."""
        yt_ps = ppool.tile([IN_FEAT, N], dt.float32, name=f"ps_{name}", tag="hopps")
        for k, (moff, mlen) in enumerate(MCHUNKS):
            nc.tensor.matmul(
                yt_ps[:], lhsT_of(k), at_sb[g][0:mlen, k * N:(k + 1) * N],
                start=(k == 0), stop=(k == len(MCHUNKS) - 1),
            )
        yt = spool.tile([IN_FEAT, N], dt.bfloat16, name=f"yt_{name}", tag="hopsb")
        nc.scalar.activation(yt[:], yt_ps[:], Copy)
        return yt

    def nat_slicer(tl):
        return lambda k: tl[0:MCHUNKS[k][1], k * IN_FEAT:(k + 1) * IN_FEAT]

    def naturalize(yt, name):
        """PE-transpose YT [66, N] -> natural tile [128, 4*66]."""
        natt = spool.tile([128, 4 * IN_FEAT], dt.bfloat16, name=f"nat_{name}", tag="natsb")
        for k, (moff, mlen) in enumerate(MCHUNKS):
            tp = ptp.tile([mlen, IN_FEAT], dt.bfloat16, name=f"tp_{name}{k}", tag="trps")
            nc.tensor.transpose(tp[:], yt[:, moff:moff + mlen], ident_sb[0:IN_FEAT, 0:IN_FEAT])
            nc.scalar.activation(natt[0:mlen, k * IN_FEAT:(k + 1) * IN_FEAT], tp[:], Copy)
        return natt

    def meta_phase(hT, lhsT_of, w_sb, b_sb, o_dim, psum_out, phase):
        """Hops + gT build + meta matmul, accumulating into psum_out [o_dim, N]."""
        for g in range(GRAPH_NUM):
            y1t = hop(lhsT_of, g, f"{phase}y1g{g}")
            piece_to_hT(hT, y1t, 1 + 2 * g)
            y1nat = naturalize(y1t, f"{phase}g{g}")
            y2t = hop(nat_slicer(y1nat), g, f"{phase}y2g{g}")
            piece_to_hT(hT, y2t, 2 + 2 * g)

        # bias matmul resets PSUM
        nc.tensor.matmul(psum_out[:], b_sb[:], embT_sb[:], start=True, stop=False)

        # gT build (fused 4-d DVE ops) + accumulate matmuls; quarter-major
        # (DMA stream order), k inner so early hT tiles are consumed first
        for q in range(D_EMB // QD):
            for k in range(KCH):
                d0 = q * QD
                c0 = d0 * KCH + k
                # out chunks c = (d0+j)*KCH + k, j=0..3 -> stride KCH*N
                out_ap = (gT[:, c0 * N:(c0 + KCH * (QD - 1) + 1) * N]
                          .rearrange("p (c n) -> p c n", n=N)[:, ::KCH, :])
                in0 = (hT[k][:, :].rearrange("p (u n) -> p u n", u=1)
                       .broadcast_to([128, QD, N]))
                in1 = (embrep_sb[:, d0 * N:(d0 + QD) * N]
                       .rearrange("p (c n) -> p c n", n=N))
                nc.vector.tensor_tensor(out_ap, in0, in1, mybir.AluOpType.mult)
                for j in range(QD):
                    c = (d0 + j) * KCH + k
                    nc.tensor.matmul(
                        psum_out[:],
                        w_sb[:, c * o_dim:(c + 1) * o_dim],
                        gT[:, c * N:(c + 1) * N],
                        start=False,
                        stop=(q == D_EMB // QD - 1 and k == KCH - 1 and j == QD - 1),
                    )

    # ================= gate phase =================
    zr_ps = pzr.tile([O_G, N], dt.float32, name="zr_ps")
    meta_phase(hT_g, nat_slicer(xsnat_sb), wg_sb, bg_sb, O_G, zr_ps, "g")
    zr_sig = hpool.tile([O_G, N], dt.float32, name="zr_sig")
    nc.scalar.activation(zr_sig[:], zr_ps[:], Sig)

    # xrsT state part: rows 64:128 of zr_sig are rT; state2 rows 64:128 = stateT
    rs_scratch = hpool.tile([O_G, N], dt.bfloat16, name="rs_scratch")
    nc.vector.tensor_mul(rs_scratch[HIDDEN:O_G, :], zr_sig[HIDDEN:O_G, :],
                         state2_sb[HIDDEN:O_G, :])
    nc.scalar.dma_start(hT_c[0][INPUT_DIM:IN_FEAT, :], rs_scratch[HIDDEN:O_G, :])

    # xrs natural (lhsT for candidate first hops)
    xrsnat = naturalize(hT_c[0][0:IN_FEAT, :], "xrs")

    # ================= candidate phase =================
    hc_ps = pzr.tile([O_C, N], dt.float32, name="hc_ps")
    meta_phase(hT_c, nat_slicer(xrsnat), wc_sb, bc_sb, O_C, hc_ps, "c")
    hc_t = hpool.tile([O_C, N], dt.float32, name="hc_t")
    nc.scalar.activation(hc_t[:], hc_ps[:], Tanh)

    # ================= output blend =================
    # h = hc + z*(state - hc)
    d1 = hpool.tile([O_C, N], dt.float32, name="d1")
    nc.vector.tensor_sub(d1[:], state2_sb[0:HIDDEN, :], hc_t[:])
    d2 = hpool.tile([O_C, N], dt.float32, name="d2")
    nc.vector.tensor_mul(d2[:], zr_sig[0:HIDDEN, :], d1[:])
    hout = hpool.tile([O_C, N], dt.float32, name="hout")
    nc.vector.tensor_add(hout[:], hc_t[:], d2[:])
    nc.sync.dma_start(d_out[:, :], hout[:])


def _build_nc():
    import concourse.tile as tile
    import concourse.mybir as mybir
    from contextlib import ExitStack
    from concourse import bacc

    nc = bacc.Bacc(trn_type="TRN2")
    with tile.TileContext(nc) as tc:
        with ExitStack() as ctx:
            _emit(nc, tc, tile, mybir, ctx)
    nc.finalize()
    return nc


def _prep_core_inputs(b, x, state, graphs, node_emb, Wg, bg, Wc, bc):
    """Host-side shard + layout prep for core b. Layouts match SBUF tiles."""
    f32 = np.float32
    at = graphs[:, b].transpose(0, 2, 1)                         # [G, N, N] = A.T
    at_pad = np.zeros((GRAPH_NUM, NPAD, N), f32)
    at_pad[:, :N, :] = at
    at_pk = (at_pad.reshape(GRAPH_NUM, 4, 128, N)
             .transpose(0, 2, 1, 3)
             .reshape(GRAPH_NUM, 128, 4 * N))                    # [G,128,(k n)]
    xs = np.concatenate([x[b], state[b]], axis=-1)               # [N, 66] f32
    xsT = np.ascontiguousarray(xs.T).astype(BF16)                # [66, N]
    xs_pad = np.zeros((NPAD, IN_FEAT), f32)
    xs_pad[:N] = xs
    xsnat = (xs_pad.reshape(4, 128, IN_FEAT)
             .transpose(1, 0, 2)
             .reshape(128, 4 * IN_FEAT))                         # [128,(k f)]
    stT = np.ascontiguousarray(state[b].T.astype(f32))           # [64, N]
    state2 = np.concatenate([stT, stT], axis=0)                  # [128, N] f32
    embT = np.ascontiguousarray(node_emb[b].T).astype(BF16)      # [16, N]
    embrep = np.ascontiguousarray(np.broadcast_to(
        embT.reshape(1, D_EMB * N), (128, D_EMB * N)))           # [128, 16N]

    def pack_w(W, o_dim):
        # W [16, 330, o] -> [128, 48*o]; chunk c=(d,k): rows i=128k+p
        Wp = np.zeros((D_EMB, I_PAD, o_dim), np.float32)
        Wp[:, :I_DIM, :] = W
        Wp = Wp.reshape(D_EMB, KCH, 128, o_dim)                  # [d,k,p,o]
        Wp = Wp.transpose(2, 0, 1, 3).reshape(128, NCH * o_dim)  # [p,(d,k,o)]
        return np.ascontiguousarray(Wp).astype(BF16)

    ident = np.eye(128, dtype=np.float32).astype(BF16)
    return {
        "at": np.ascontiguousarray(at_pk).astype(BF16),
        "xsT": xsT,
        "xsnat": np.ascontiguousarray(xsnat).astype(BF16),
        "state2": state2,
        "embT": embT,
        "embrep": embrep,
        "wg": pack_w(Wg, O_G),
        "wc": pack_w(Wc, O_C),
        "bg": bg.astype(BF16),
        "bc": bc.astype(BF16),
        "ident": ident,
    }


def kernel_with_results(x, state, graphs, node_emb, Wg, bg, Wc, bc, trace=False):
    from concourse.bass_utils import run_bass_kernel_spmd

    x = np.asarray(x, np.float32)
    state = np.asarray(state, np.float32)
    graphs = np.asarray(graphs, np.float32)
    node_emb = np.asarray(node_emb, np.float32)
    Wg = np.asarray(Wg, np.float32)
    bg = np.asarray(bg, np.float32)
    Wc = np.asarray(Wc, np.float32)
    bc = np.asarray(bc, np.float32)

    if "nc" not in _CACHE:
        _CACHE["nc"] = _build_nc()
    nc = _CACHE["nc"]

    in_maps = [
        _prep_core_inputs(b, x, state, graphs, node_emb, Wg, bg, Wc, bc)
        for b in range(B)
    ]
    res = run_bass_kernel_spmd(nc, in_maps, core_ids=list(range(B)), trace=trace)
    out = np.stack(
        [np.ascontiguousarray(res.results[b]["out"].T) for b in range(B)], axis=0
    )  # [B, N, HIDDEN] f32
    return out, res


def kernel(**inputs):
    out, _ = kernel_with_results(**inputs)
    return out
